# revision 46
# baseline (speedup 1.0000x reference)
"""Windowed local self-attention (CrossAttention module with the context-
overwrite bug faithfully reproduced) on 8 Trainium2 NeuronCores.

Full-input contract: kernel(**inputs) takes the unsharded tensors and
returns the full (4, 4096, 1024) output. Internally the 64 independent
windows of 256 tokens are data-parallel sharded 8-per-core; the four
projection weights are broadcast to every core. No collectives needed.

All matmul operands are bf16 (host-cast): 1 cycle/row on the PE, half
the SBUF/DMA traffic of fp32, far less PE power than fp32 HIGH mode
(which triggered 50% periodic throttling in the fp32r version). PSUM
accumulation, softmax normalization and the final output stay fp32.

Key structure:
- X is transposed on the HOST: the kernel DMAs X^T tiles straight into
  SBUF, so no PE transposes / identity preamble at all.
- Windows processed in PAIRS (512 tokens): every projection/output
  matmul streams the max 512 moving rows, hiding LDWEIGHTS.
- V is stored interleaved per head as [v_h (64) | ones (64)]; the AV
  matmul then emits the attention numerator on rows 0-63 AND the
  softmax denominator (replicated) on rows 64-127 -- no row-sum matmul.
- The AV results of a head pair share one PSUM bank -> one reciprocal
  per two heads.
- Software pipelining: the attention phase of pair p is DVE/ACT-paced,
  so the projection chains of pair p+1 (and pair p's output-projection
  chains) are interleaved into its step loop to keep the PE streaming.

Per-core pipeline (window = 256 tokens, H=16 heads, DH=64):
  qT = Wq.T @ X.T   (lhsT=Wq tiles,  rhs=XT)          [o, i]
  kT = Wk.T @ X.T                                      [o, i]
  v  = X @ Wv       (lhsT=XT tiles,  rhs=Wv)           [j, v|1]
  per (window, head):
    simT = kT_h.T-free @ qT_h   -> [j, i] in PSUM     (j on partitions)
    es   = exp(0.125 * simT)    (ACT, PSUM->SBUF, bf16)
    av   = [v_h|1].T-free @ es  -> [128, i] PSUM
    rS   = 1/S   (one DVE reciprocal per head pair, full PSUM bank)
    o2T  = o2u * rS             (DVE, bf16 [o, i] SBUF)
  Y = o2T.T @ Wo       (lhsT=o2T tiles, rhs=Wo; zero bias added host-side)
"""

import numpy as np
import ml_dtypes

import concourse.bass as bass
import concourse.mybir as mybir
import concourse.tile as tile
from concourse import bacc, bass_utils
from concourse.bass_interp import get_hw_module

H = 16
DH = 64
WIN = 256
D = 1024
B = 4
N = 4096
N_CORES = 8
N_WIN_TOTAL = B * N // WIN          # 64
N_WIN = N_WIN_TOTAL // N_CORES      # 8 windows per core
TOK = N_WIN * WIN                   # 2048 token rows per core
PAIR = 2 * WIN                      # 512 tokens per window pair
SCALE = DH ** -0.5

F32 = mybir.dt.float32
BF16 = mybir.dt.bfloat16


def _body(tc, xqT, wq, wk, wv, wo, out, n_win):
    nc = tc.nc
    from contextlib import ExitStack

    n_pair = n_win // 2

    with ExitStack() as ctx:
        singles = ctx.enter_context(tc.tile_pool(name="singles", bufs=1))
        acts = ctx.enter_context(tc.tile_pool(name="acts", bufs=1))
        heads = ctx.enter_context(tc.tile_pool(name="heads", bufs=3))
        ypool = ctx.enter_context(tc.tile_pool(name="ypool", bufs=2))
        psA = ctx.enter_context(tc.tile_pool(name="psA", bufs=3, space="PSUM"))
        psS = ctx.enter_context(tc.tile_pool(name="psS", bufs=3, space="PSUM"))
        psV = ctx.enter_context(tc.tile_pool(name="psV", bufs=2, space="PSUM"))

        def emit_xt_dma(wp):
            halves = make_xt(wp)
            for hf in range(2):
                dma_xt_half(halves[hf], wp, hf)
            return halves

        # weights in half-tiles too; DMA enqueue order puts the chain-gating
        # halves (xt0_a, wq_a, wk_a) first
        wsb = {}
        wdram = {"wq": wq, "wk": wk, "wv": wv, "wo": wo}
        for name in ("wq", "wk", "wv", "wo"):
            wsb[name] = [
                singles.tile([128, 4 * D], BF16, tag=f"{name}{hf}",
                             name=f"sb_{name}_{hf}")
                for hf in range(2)
            ]

        def dma_w_half(name, hf):
            nc.sync.dma_start(
                wsb[name][hf][:].rearrange("p (kt c) -> p kt c", kt=4),
                wdram[name][hf * 512:(hf + 1) * 512, :]
                .rearrange("(kt p) c -> p kt c", p=128),
            )

        def make_xt(wp):
            return [acts.tile([128, 4 * 512], BF16, tag=f"xt{hf}", bufs=2,
                              name=f"xt_{wp}_{hf}") for hf in range(2)]

        def dma_xt_half(t, wp, hf):
            nc.sync.dma_start(
                t[:].rearrange("p (dt t) -> p dt t", dt=4),
                xqT[hf * 512:(hf + 1) * 512, wp * PAIR:(wp + 1) * PAIR]
                .rearrange("(dt p) t -> p dt t", p=128),
            )

        # pair 0's first chains are the q chains (xt + wq only): order the
        # transfers so those 3 MB land first, wk streams while q chains run
        xt0 = make_xt(0)
        dma_xt_half(xt0[0], 0, 0)
        dma_xt_half(xt0[1], 0, 1)
        for name in ("wq", "wk", "wv", "wo"):
            for hf in range(2):
                dma_w_half(name, hf)

        # v buffers: pair parity x window -> 4 buffers; per-head layout
        # [v_h (64 cols) | ones (64 cols)] so AV' yields sums on rows 64+.
        v2b = []
        for i in range(4):
            t = singles.tile([128, 2 * H * 128], BF16, name=f"v2_{i}")
            ones_view = t[:].rearrange("p (j h c) -> p j h c", j=2, h=H)[:, :, :, DH:]
            nc.gpsimd.memset(ones_view, 1.0)
            v2b.append(t)

        def proj_chains(wp, xt):
            """qT/kT/v chains for pair wp as a list of zero-arg closures."""
            proj = {}
            for pname in ("qT", "kT"):
                proj[pname] = acts.tile([128, 8 * 512], BF16, tag=pname,
                                        bufs=2, name=f"{pname}_{wp}")
            chains = []
            for ot in range(8):
                for pname, wname in (("qT", "wq"), ("kT", "wk")):
                    def qk_chain(ot=ot, pname=pname, wname=wname):
                        pq = psA.tile([128, 512], F32, tag="acc",
                                      name=f"pq_{wp}_{pname}_{ot}")
                        for kt in range(8):
                            hf, kl = kt // 4, kt % 4
                            nc.tensor.matmul(
                                pq[:],
                                wsb[wname][hf][:, kl * D + ot * 128:
                                               kl * D + (ot + 1) * 128],
                                xt[hf][:, kl * 512:(kl + 1) * 512],
                                start=(kt == 0),
                                stop=(kt == 7),
                            )
                        nc.vector.tensor_copy(
                            proj[pname][:, ot * 512:(ot + 1) * 512], pq[:]
                        )
                    chains.append(qk_chain)
            for tt in range(4):
                for oc in range(2):
                    def v_chain(tt=tt, oc=oc):
                        wl, jt = tt // 2, tt % 2
                        pv = psA.tile([128, 512], F32, tag="acc",
                                      name=f"pv_{wp}_{tt}_{oc}")
                        for kt in range(8):
                            hf, kl = kt // 4, kt % 4
                            nc.tensor.matmul(
                                pv[:],
                                xt[hf][:, kl * 512 + tt * 128:
                                       kl * 512 + (tt + 1) * 128],
                                wsb["wv"][hf][:, kl * D + oc * 512:
                                              kl * D + (oc + 1) * 512],
                                start=(kt == 0),
                                stop=(kt == 7),
                            )
                        vdst = v2b[(wp % 2) * 2 + wl]
                        dsl = vdst[:, jt * H * 128 + oc * 8 * 128:
                                   jt * H * 128 + (oc + 1) * 8 * 128]
                        nc.scalar.copy(
                            dsl.rearrange("p (h c) -> p h c", h=8)[:, :, 0:DH],
                            pv[:],
                        )
                    chains.append(v_chain)
            return proj, chains

        def emit_y_group(wp, o2T, it, ec):
            row0 = wp * PAIR
            py = psA.tile([128, 512], F32, tag="acc", name=f"py_{wp}_{it}_{ec}")
            for kt2 in range(8):
                hf, kl = kt2 // 4, kt2 % 4
                nc.tensor.matmul(
                    py[:],
                    o2T[:, kt2 * 512 + it * 128:kt2 * 512 + (it + 1) * 128],
                    wsb["wo"][hf][:, kl * D + ec * 512:kl * D + (ec + 1) * 512],
                    start=(kt2 == 0),
                    stop=(kt2 == 7),
                )
            y_sb = ypool.tile([128, 512], BF16, tag="y", name=f"y_{wp}_{it}_{ec}")
            nc.vector.tensor_copy(y_sb[:], py[:])
            nc.sync.dma_start(
                out[row0 + it * 128:row0 + (it + 1) * 128, ec * 512:(ec + 1) * 512],
                y_sb[:],
            )

        def attention(wp, qT, kT, o2T, extra):
            """32 (window, head) steps; `extra` chains are paced through the
            step loop to keep the PE streaming while DVE/ACT normalize."""
            # heads paired (p, p+8): same o2T rows, column blocks 4*512
            # apart -> the pair's two normalize-muls merge into one op
            steps = [(wl, p + 8 * sub) for wl in range(2)
                     for p in range(8) for sub in range(2)]
            es_t = {}
            av_t = {}

            def emit_sim(i):
                wl, h = steps[i]
                prow = (h % 2) * 64
                ocol = (h // 2) * 512 + wl * WIN
                qh = qT[prow:prow + 64, ocol:ocol + WIN]
                kh = kT[prow:prow + 64, ocol:ocol + WIN]
                ps_sim = psS.tile([128, 512], F32, tag="sim",
                                  name=f"sim_{wp}_{wl}_{h}")
                for jt in range(2):
                    nc.tensor.matmul(
                        ps_sim[:, jt * WIN:(jt + 1) * WIN],
                        kh[:, jt * 128:(jt + 1) * 128],
                        qh,
                        start=True,
                        stop=True,
                    )
                e = heads.tile([128, 512], BF16, tag="es", bufs=4,
                               name=f"es_{wp}_{wl}_{h}")
                nc.scalar.activation(
                    e[:], ps_sim[:], mybir.ActivationFunctionType.Exp, scale=SCALE
                )
                es_t[i] = e

            def emit_av(i):
                wl, h = steps[i]
                if h < 8:
                    av_t[i // 2] = psV.tile([128, 512], F32, tag="av",
                                            name=f"av_{wp}_{wl}_{h}")
                av2 = av_t[i // 2]
                c0 = (h // 8) * WIN
                es = es_t.pop(i)
                for jt in range(2):
                    nc.tensor.matmul(
                        av2[:, c0:c0 + WIN],
                        v2b[(wp % 2) * 2 + wl][:, (jt * H + h) * 128:
                                               (jt * H + h + 1) * 128],
                        es[:, jt * WIN:(jt + 1) * WIN],
                        start=(jt == 0),
                        stop=(jt == 1),
                    )

            def emit_epilogue(p):
                # heads h0, h0+8 of window wl share av bank p: one recip and
                # ONE strided mul (same o2T rows, blocks h0//2 and h0//2+4)
                av2 = av_t.pop(p)
                wl, h0 = steps[2 * p]
                rs = heads.tile([128, 512], F32, tag="rs", name=f"rs_{wp}_{p}")
                nc.vector.reciprocal_approx_fast(rs[:], av2[:])
                r0 = (h0 % 2) * 64
                dst = o2T[r0:r0 + 64, :] \
                    .rearrange("p (b c) -> p b c", c=512) \
                    [:, (h0 // 2)::4, wl * WIN:(wl + 1) * WIN]
                nc.vector.tensor_mul(
                    dst,
                    av2[0:64, :].rearrange("p (b c) -> p b c", b=2),
                    rs[64:128, :].rearrange("p (b c) -> p b c", b=2),
                )

            n_extra = len(extra)
            ch_i = 0
            for i in range(3):
                emit_sim(i)
            for i in range(len(steps)):
                if i + 3 < len(steps):
                    emit_sim(i + 3)
                emit_av(i)
                if i % 2 == 1:
                    emit_epilogue(i // 2)
                if i in (15, 17):
                    # window 0 fully normalized at step 15: flow its Y groups
                    g = 0 if i == 15 else 1
                    emit_y_group(wp, o2T, g, 0)
                    emit_y_group(wp, o2T, g, 1)
                while ch_i * len(steps) < n_extra * (i + 1):
                    extra[ch_i]()
                    ch_i += 1
            for g in range(2, 4):
                emit_y_group(wp, o2T, g, 0)
                emit_y_group(wp, o2T, g, 1)

        proj, chains0 = proj_chains(0, xt0)
        # pair 0 q chains split into half-K segments: the A segments only
        # need xt half 0 + wq half 0, so the PE starts before the rest of
        # the weight DMA lands; B segments continue the same PSUM groups
        def q_seg(ot, hf, pq):
            for kl in range(4):
                kt = hf * 4 + kl
                nc.tensor.matmul(
                    pq[:],
                    wsb["wq"][hf][:, kl * D + ot * 128:kl * D + (ot + 1) * 128],
                    xt0[hf][:, kl * 512:(kl + 1) * 512],
                    start=(kt == 0),
                    stop=(kt == 7),
                )
        pq_t = {}
        def q_a(ot):
            pq_t[ot] = psA.tile([128, 512], F32, tag="acc", name=f"pq0_qT_{ot}")
            q_seg(ot, 0, pq_t[ot])
        def q_b(ot):
            pq = pq_t.pop(ot)
            q_seg(ot, 1, pq)
            nc.vector.tensor_copy(proj["qT"][:, ot * 512:(ot + 1) * 512], pq[:])
        for ot in range(3):
            q_a(ot)
        for ot in range(8):
            q_b(ot)
            if ot + 3 < 8:
                q_a(ot + 3)
        for ch in chains0[1::2][:8] + chains0[16:]:
            ch()
        for wp in range(n_pair):
            o2T = acts.tile([128, 8 * 512], BF16, tag="o2T", bufs=2,
                            name=f"o2T_{wp}")
            if wp + 1 < n_pair:
                xt_next = emit_xt_dma(wp + 1)
                proj_next, chains_next = proj_chains(wp + 1, xt_next)
            else:
                proj_next, chains_next = None, []
            attention(wp, proj["qT"], proj["kT"], o2T, chains_next)
            proj = proj_next


_CACHE = {}


def _build(n_win=N_WIN):
    key = n_win
    if key in _CACHE:
        return _CACHE[key]
    tok = n_win * WIN
    nc = bacc.Bacc(
        "TRN2", target_bir_lowering=False, debug=False, num_devices=N_CORES
    )
    xqT = nc.dram_tensor("xqT", [D, tok], BF16, kind="ExternalInput").ap()
    wq = nc.dram_tensor("Wq", [D, D], BF16, kind="ExternalInput").ap()
    wk = nc.dram_tensor("Wk", [D, D], BF16, kind="ExternalInput").ap()
    wv = nc.dram_tensor("Wv", [D, D], BF16, kind="ExternalInput").ap()
    wo = nc.dram_tensor("Wo", [D, D], BF16, kind="ExternalInput").ap()
    out = nc.dram_tensor("out", [tok, D], BF16, kind="ExternalOutput").ap()
    with tile.TileContext(nc) as tc:
        _body(tc, xqT, wq, wk, wv, wo, out, n_win)
    nc.compile()
    nc.m = get_hw_module(nc.m)
    _CACHE[key] = nc
    return nc


def run(query, Wq, Wk, Wv, Wo, bo, n_win=N_WIN, **spmd_kwargs):
    nc = _build(n_win)
    tok = n_win * WIN
    bf = ml_dtypes.bfloat16
    q2 = np.asarray(query, dtype=np.float32).reshape(-1, D).astype(bf)
    weights = {
        "Wq": np.ascontiguousarray(np.asarray(Wq, np.float32).astype(bf)),
        "Wk": np.ascontiguousarray(np.asarray(Wk, np.float32).astype(bf)),
        "Wv": np.ascontiguousarray(np.asarray(Wv, np.float32).astype(bf)),
        "Wo": np.ascontiguousarray(np.asarray(Wo, np.float32).astype(bf)),
    }
    in_maps = []
    for c in range(N_CORES):
        m = {"xqT": np.ascontiguousarray(q2[c * TOK:c * TOK + tok].T)}
        m.update(weights)
        in_maps.append(m)
    try:
        res = bass_utils.run_bass_kernel_spmd(
            nc, in_maps, core_ids=list(range(N_CORES)), **spmd_kwargs
        )
    except Exception:
        # transient NRT_EXEC_UNIT_UNRECOVERABLE wedges clear on retry
        res = bass_utils.run_bass_kernel_spmd(
            nc, in_maps, core_ids=list(range(N_CORES)), **spmd_kwargs
        )
    outs = [np.asarray(res.results[c]["out"]).astype(np.float32)
            for c in range(N_CORES)]
    return outs, res


def kernel(query, context, Wq, Wk, Wv, Wo, bo):
    outs, _ = run(query, Wq, Wk, Wv, Wo, bo)
    y = np.concatenate(outs, axis=0).reshape(B, N, D)
    bo = np.asarray(bo, np.float32)
    if bo.any():
        y = y + bo  # bias is structurally zero for this problem; host-add keeps exactness
    return y.astype(np.float32)


# revision 47
# speedup vs baseline: 1.0035x; 1.0035x over previous
"""Windowed local self-attention (CrossAttention module with the context-
overwrite bug faithfully reproduced) on 8 Trainium2 NeuronCores.

Full-input contract: kernel(**inputs) takes the unsharded tensors and
returns the full (4, 4096, 1024) output. Internally the 64 independent
windows of 256 tokens are data-parallel sharded 8-per-core; the four
projection weights are broadcast to every core. No collectives needed.

All matmul operands are bf16 (host-cast): 1 cycle/row on the PE, half
the SBUF/DMA traffic of fp32, far less PE power than fp32 HIGH mode
(which triggered 50% periodic throttling in the fp32r version). PSUM
accumulation, softmax normalization and the final output stay fp32.

Key structure:
- X is transposed on the HOST: the kernel DMAs X^T tiles straight into
  SBUF, so no PE transposes / identity preamble at all.
- Windows processed in PAIRS (512 tokens): every projection/output
  matmul streams the max 512 moving rows, hiding LDWEIGHTS.
- V is stored interleaved per head as [v_h (64) | ones (64)]; the AV
  matmul then emits the attention numerator on rows 0-63 AND the
  softmax denominator (replicated) on rows 64-127 -- no row-sum matmul.
- The AV results of a head pair share one PSUM bank -> one reciprocal
  per two heads.
- Software pipelining: the attention phase of pair p is DVE/ACT-paced,
  so the projection chains of pair p+1 (and pair p's output-projection
  chains) are interleaved into its step loop to keep the PE streaming.

Per-core pipeline (window = 256 tokens, H=16 heads, DH=64):
  qT = Wq.T @ X.T   (lhsT=Wq tiles,  rhs=XT)          [o, i]
  kT = Wk.T @ X.T                                      [o, i]
  v  = X @ Wv       (lhsT=XT tiles,  rhs=Wv)           [j, v|1]
  per (window, head):
    simT = kT_h.T-free @ qT_h   -> [j, i] in PSUM     (j on partitions)
    es   = exp(0.125 * simT)    (ACT, PSUM->SBUF, bf16)
    av   = [v_h|1].T-free @ es  -> [128, i] PSUM
    rS   = 1/S   (one DVE reciprocal per head pair, full PSUM bank)
    o2T  = o2u * rS             (DVE, bf16 [o, i] SBUF)
  Y = o2T.T @ Wo       (lhsT=o2T tiles, rhs=Wo; zero bias added host-side)
"""

import numpy as np
import ml_dtypes

import concourse.bass as bass
import concourse.mybir as mybir
import concourse.tile as tile
from concourse import bacc, bass_utils
from concourse.bass_interp import get_hw_module

H = 16
DH = 64
WIN = 256
D = 1024
B = 4
N = 4096
N_CORES = 8
N_WIN_TOTAL = B * N // WIN          # 64
N_WIN = N_WIN_TOTAL // N_CORES      # 8 windows per core
TOK = N_WIN * WIN                   # 2048 token rows per core
PAIR = 2 * WIN                      # 512 tokens per window pair
SCALE = DH ** -0.5

F32 = mybir.dt.float32
BF16 = mybir.dt.bfloat16


def _body(tc, xqT, wq, wk, wv, wo, out, n_win):
    nc = tc.nc
    from contextlib import ExitStack

    n_pair = n_win // 2

    with ExitStack() as ctx:
        singles = ctx.enter_context(tc.tile_pool(name="singles", bufs=1))
        acts = ctx.enter_context(tc.tile_pool(name="acts", bufs=1))
        heads = ctx.enter_context(tc.tile_pool(name="heads", bufs=3))
        ypool = ctx.enter_context(tc.tile_pool(name="ypool", bufs=2))
        psA = ctx.enter_context(tc.tile_pool(name="psA", bufs=3, space="PSUM"))
        psS = ctx.enter_context(tc.tile_pool(name="psS", bufs=3, space="PSUM"))
        psV = ctx.enter_context(tc.tile_pool(name="psV", bufs=2, space="PSUM"))

        def emit_xt_dma(wp):
            halves = make_xt(wp)
            for hf in range(2):
                dma_xt_half(halves[hf], wp, hf)
            return halves

        # weights in half-tiles too; DMA enqueue order puts the chain-gating
        # halves (xt0_a, wq_a, wk_a) first
        wsb = {}
        wdram = {"wq": wq, "wk": wk, "wv": wv, "wo": wo}
        for name in ("wq", "wk", "wv", "wo"):
            wsb[name] = [
                singles.tile([128, 4 * D], BF16, tag=f"{name}{hf}",
                             name=f"sb_{name}_{hf}")
                for hf in range(2)
            ]

        def dma_w_half(name, hf):
            nc.sync.dma_start(
                wsb[name][hf][:].rearrange("p (kt c) -> p kt c", kt=4),
                wdram[name][hf * 512:(hf + 1) * 512, :]
                .rearrange("(kt p) c -> p kt c", p=128),
            )

        def make_xt(wp):
            return [acts.tile([128, 4 * 512], BF16, tag=f"xt{hf}", bufs=2,
                              name=f"xt_{wp}_{hf}") for hf in range(2)]

        def dma_xt_half(t, wp, hf):
            nc.sync.dma_start(
                t[:].rearrange("p (dt t) -> p dt t", dt=4),
                xqT[hf * 512:(hf + 1) * 512, wp * PAIR:(wp + 1) * PAIR]
                .rearrange("(dt p) t -> p dt t", p=128),
            )

        # pair 0's first chains are the q chains (xt + wq only): order the
        # transfers so those 3 MB land first, wk streams while q chains run
        xt0 = make_xt(0)
        dma_xt_half(xt0[0], 0, 0)
        dma_xt_half(xt0[1], 0, 1)
        for name in ("wq", "wk", "wv", "wo"):
            for hf in range(2):
                dma_w_half(name, hf)

        # v buffers: pair parity x window -> 4 buffers; per-head layout
        # [v_h (64 cols) | ones (64 cols)] so AV' yields sums on rows 64+.
        v2b = []
        for i in range(4):
            t = singles.tile([128, 2 * H * 128], BF16, name=f"v2_{i}")
            ones_view = t[:].rearrange("p (j h c) -> p j h c", j=2, h=H)[:, :, :, DH:]
            nc.gpsimd.memset(ones_view, 1.0)
            v2b.append(t)

        def proj_chains(wp, xt):
            """qT/kT/v chains for pair wp as a list of zero-arg closures."""
            proj = {}
            for pname in ("qT", "kT"):
                proj[pname] = acts.tile([128, 8 * 512], BF16, tag=pname,
                                        bufs=2, name=f"{pname}_{wp}")
            chains = []
            for ot in range(8):
                for pname, wname in (("qT", "wq"), ("kT", "wk")):
                    def qk_chain(ot=ot, pname=pname, wname=wname):
                        pq = psA.tile([128, 512], F32, tag="acc",
                                      name=f"pq_{wp}_{pname}_{ot}")
                        for kt in range(8):
                            hf, kl = kt // 4, kt % 4
                            nc.tensor.matmul(
                                pq[:],
                                wsb[wname][hf][:, kl * D + ot * 128:
                                               kl * D + (ot + 1) * 128],
                                xt[hf][:, kl * 512:(kl + 1) * 512],
                                start=(kt == 0),
                                stop=(kt == 7),
                            )
                        cp = (nc.vector.tensor_copy if pname == "qT"
                              else nc.scalar.copy)
                        cp(proj[pname][:, ot * 512:(ot + 1) * 512], pq[:])
                    chains.append(qk_chain)
            for tt in range(4):
                for oc in range(2):
                    def v_chain(tt=tt, oc=oc):
                        wl, jt = tt // 2, tt % 2
                        pv = psA.tile([128, 512], F32, tag="acc",
                                      name=f"pv_{wp}_{tt}_{oc}")
                        for kt in range(8):
                            hf, kl = kt // 4, kt % 4
                            nc.tensor.matmul(
                                pv[:],
                                xt[hf][:, kl * 512 + tt * 128:
                                       kl * 512 + (tt + 1) * 128],
                                wsb["wv"][hf][:, kl * D + oc * 512:
                                              kl * D + (oc + 1) * 512],
                                start=(kt == 0),
                                stop=(kt == 7),
                            )
                        vdst = v2b[(wp % 2) * 2 + wl]
                        dsl = vdst[:, jt * H * 128 + oc * 8 * 128:
                                   jt * H * 128 + (oc + 1) * 8 * 128]
                        nc.scalar.copy(
                            dsl.rearrange("p (h c) -> p h c", h=8)[:, :, 0:DH],
                            pv[:],
                        )
                    chains.append(v_chain)
            return proj, chains

        def emit_y_group(wp, o2T, it, ec):
            row0 = wp * PAIR
            py = psA.tile([128, 512], F32, tag="acc", name=f"py_{wp}_{it}_{ec}")
            for kt2 in range(8):
                hf, kl = kt2 // 4, kt2 % 4
                nc.tensor.matmul(
                    py[:],
                    o2T[:, kt2 * 512 + it * 128:kt2 * 512 + (it + 1) * 128],
                    wsb["wo"][hf][:, kl * D + ec * 512:kl * D + (ec + 1) * 512],
                    start=(kt2 == 0),
                    stop=(kt2 == 7),
                )
            y_sb = ypool.tile([128, 512], BF16, tag="y", name=f"y_{wp}_{it}_{ec}")
            nc.vector.tensor_copy(y_sb[:], py[:])
            nc.sync.dma_start(
                out[row0 + it * 128:row0 + (it + 1) * 128, ec * 512:(ec + 1) * 512],
                y_sb[:],
            )

        def attention(wp, qT, kT, o2T, extra):
            """32 (window, head) steps; `extra` chains are paced through the
            step loop to keep the PE streaming while DVE/ACT normalize."""
            # heads paired (p, p+8): same o2T rows, column blocks 4*512
            # apart -> the pair's two normalize-muls merge into one op
            steps = [(wl, p + 8 * sub) for wl in range(2)
                     for p in range(8) for sub in range(2)]
            es_t = {}
            av_t = {}

            def emit_sim(i):
                wl, h = steps[i]
                prow = (h % 2) * 64
                ocol = (h // 2) * 512 + wl * WIN
                qh = qT[prow:prow + 64, ocol:ocol + WIN]
                kh = kT[prow:prow + 64, ocol:ocol + WIN]
                ps_sim = psS.tile([128, 512], F32, tag="sim",
                                  name=f"sim_{wp}_{wl}_{h}")
                for jt in range(2):
                    nc.tensor.matmul(
                        ps_sim[:, jt * WIN:(jt + 1) * WIN],
                        kh[:, jt * 128:(jt + 1) * 128],
                        qh,
                        start=True,
                        stop=True,
                    )
                e = heads.tile([128, 512], BF16, tag="es", bufs=4,
                               name=f"es_{wp}_{wl}_{h}")
                nc.scalar.activation(
                    e[:], ps_sim[:], mybir.ActivationFunctionType.Exp, scale=SCALE
                )
                es_t[i] = e

            def emit_av(i):
                wl, h = steps[i]
                if h < 8:
                    av_t[i // 2] = psV.tile([128, 512], F32, tag="av",
                                            name=f"av_{wp}_{wl}_{h}")
                av2 = av_t[i // 2]
                c0 = (h // 8) * WIN
                es = es_t.pop(i)
                for jt in range(2):
                    nc.tensor.matmul(
                        av2[:, c0:c0 + WIN],
                        v2b[(wp % 2) * 2 + wl][:, (jt * H + h) * 128:
                                               (jt * H + h + 1) * 128],
                        es[:, jt * WIN:(jt + 1) * WIN],
                        start=(jt == 0),
                        stop=(jt == 1),
                    )

            def emit_epilogue(p):
                # heads h0, h0+8 of window wl share av bank p: one recip and
                # ONE strided mul (same o2T rows, blocks h0//2 and h0//2+4)
                av2 = av_t.pop(p)
                wl, h0 = steps[2 * p]
                rs = heads.tile([128, 512], F32, tag="rs", name=f"rs_{wp}_{p}")
                nc.vector.reciprocal_approx_fast(rs[:], av2[:])
                r0 = (h0 % 2) * 64
                dst = o2T[r0:r0 + 64, :] \
                    .rearrange("p (b c) -> p b c", c=512) \
                    [:, (h0 // 2)::4, wl * WIN:(wl + 1) * WIN]
                nc.vector.tensor_mul(
                    dst,
                    av2[0:64, :].rearrange("p (b c) -> p b c", b=2),
                    rs[64:128, :].rearrange("p (b c) -> p b c", b=2),
                )

            n_extra = len(extra)
            ch_i = 0
            for i in range(3):
                emit_sim(i)
            for i in range(len(steps)):
                if i + 3 < len(steps):
                    emit_sim(i + 3)
                emit_av(i)
                if i % 2 == 1:
                    emit_epilogue(i // 2)
                if i in (15, 17):
                    # window 0 fully normalized at step 15: flow its Y groups
                    g = 0 if i == 15 else 1
                    emit_y_group(wp, o2T, g, 0)
                    emit_y_group(wp, o2T, g, 1)
                while ch_i * len(steps) < n_extra * (i + 1):
                    extra[ch_i]()
                    ch_i += 1
            for g in range(2, 4):
                emit_y_group(wp, o2T, g, 0)
                emit_y_group(wp, o2T, g, 1)

        proj, chains0 = proj_chains(0, xt0)
        # pair 0 q chains split into half-K segments: the A segments only
        # need xt half 0 + wq half 0, so the PE starts before the rest of
        # the weight DMA lands; B segments continue the same PSUM groups
        def q_seg(ot, hf, pq):
            for kl in range(4):
                kt = hf * 4 + kl
                nc.tensor.matmul(
                    pq[:],
                    wsb["wq"][hf][:, kl * D + ot * 128:kl * D + (ot + 1) * 128],
                    xt0[hf][:, kl * 512:(kl + 1) * 512],
                    start=(kt == 0),
                    stop=(kt == 7),
                )
        pq_t = {}
        def q_a(ot):
            pq_t[ot] = psA.tile([128, 512], F32, tag="acc", name=f"pq0_qT_{ot}")
            q_seg(ot, 0, pq_t[ot])
        def q_b(ot):
            pq = pq_t.pop(ot)
            q_seg(ot, 1, pq)
            nc.vector.tensor_copy(proj["qT"][:, ot * 512:(ot + 1) * 512], pq[:])
        for ot in range(3):
            q_a(ot)
        for ot in range(8):
            q_b(ot)
            if ot + 3 < 8:
                q_a(ot + 3)
        for ch in chains0[1::2][:8] + chains0[16:]:
            ch()
        for wp in range(n_pair):
            o2T = acts.tile([128, 8 * 512], BF16, tag="o2T", bufs=2,
                            name=f"o2T_{wp}")
            if wp + 1 < n_pair:
                xt_next = emit_xt_dma(wp + 1)
                proj_next, chains_next = proj_chains(wp + 1, xt_next)
            else:
                proj_next, chains_next = None, []
            attention(wp, proj["qT"], proj["kT"], o2T, chains_next)
            proj = proj_next


_CACHE = {}


def _build(n_win=N_WIN):
    key = n_win
    if key in _CACHE:
        return _CACHE[key]
    tok = n_win * WIN
    nc = bacc.Bacc(
        "TRN2", target_bir_lowering=False, debug=False, num_devices=N_CORES
    )
    xqT = nc.dram_tensor("xqT", [D, tok], BF16, kind="ExternalInput").ap()
    wq = nc.dram_tensor("Wq", [D, D], BF16, kind="ExternalInput").ap()
    wk = nc.dram_tensor("Wk", [D, D], BF16, kind="ExternalInput").ap()
    wv = nc.dram_tensor("Wv", [D, D], BF16, kind="ExternalInput").ap()
    wo = nc.dram_tensor("Wo", [D, D], BF16, kind="ExternalInput").ap()
    out = nc.dram_tensor("out", [tok, D], BF16, kind="ExternalOutput").ap()
    with tile.TileContext(nc) as tc:
        _body(tc, xqT, wq, wk, wv, wo, out, n_win)
    nc.compile()
    nc.m = get_hw_module(nc.m)
    _CACHE[key] = nc
    return nc


def run(query, Wq, Wk, Wv, Wo, bo, n_win=N_WIN, **spmd_kwargs):
    nc = _build(n_win)
    tok = n_win * WIN
    bf = ml_dtypes.bfloat16
    q2 = np.asarray(query, dtype=np.float32).reshape(-1, D).astype(bf)
    weights = {
        "Wq": np.ascontiguousarray(np.asarray(Wq, np.float32).astype(bf)),
        "Wk": np.ascontiguousarray(np.asarray(Wk, np.float32).astype(bf)),
        "Wv": np.ascontiguousarray(np.asarray(Wv, np.float32).astype(bf)),
        "Wo": np.ascontiguousarray(np.asarray(Wo, np.float32).astype(bf)),
    }
    in_maps = []
    for c in range(N_CORES):
        m = {"xqT": np.ascontiguousarray(q2[c * TOK:c * TOK + tok].T)}
        m.update(weights)
        in_maps.append(m)
    try:
        res = bass_utils.run_bass_kernel_spmd(
            nc, in_maps, core_ids=list(range(N_CORES)), **spmd_kwargs
        )
    except Exception:
        # transient NRT_EXEC_UNIT_UNRECOVERABLE wedges clear on retry
        res = bass_utils.run_bass_kernel_spmd(
            nc, in_maps, core_ids=list(range(N_CORES)), **spmd_kwargs
        )
    outs = [np.asarray(res.results[c]["out"]).astype(np.float32)
            for c in range(N_CORES)]
    return outs, res


def kernel(query, context, Wq, Wk, Wv, Wo, bo):
    outs, _ = run(query, Wq, Wk, Wv, Wo, bo)
    y = np.concatenate(outs, axis=0).reshape(B, N, D)
    bo = np.asarray(bo, np.float32)
    if bo.any():
        y = y + bo  # bias is structurally zero for this problem; host-add keeps exactness
    return y.astype(np.float32)
